# revision 1
# baseline (speedup 1.0000x reference)
"""Multi-head attention (N=2048, D=1024, H=16) on 8 TRN2 NeuronCores.

Sharding: tensor-parallel over heads (2 heads / core). x is replicated
(pre-transposed + pre-cast on host), each core computes QKV / scores /
softmax / PV / out-proj for its 2 heads, producing a partial (N, D)
projection output in fp16. The all-reduce over cores is the host-side
f64 sum of the 8 partials (+ b_proj), cast back to f32.

Device inputs (per core):
  xT      (D, N)    bf16 : x transposed (host prep)
  wqkvT   (D, 384)  bf16 : [Wq.T | Wk.T | Wv.T] column slices for 2 heads
  wpT     (128, D)  f32r : w_proj[:, core_cols].T
  bqkv    (128, 3)  f32  : [bq | bk | bv] slices
  out_part(N, D)    f16  : partial projection output

Per-core pipeline (streamed; emission order sets Tile priorities):
  QKV   bf16 matmuls, j-sliced and interleaved with block-0 attention so
        scores start while x is still streaming in -> Q.T/K.T (head-dim
        on partitions, bf16) and V.T
  V.T   --PE transpose--> V (seq on partitions) with ones columns
  scores S.T = K.T^T @ Q.T per head; both heads packed into the PE array
        via 64-row tiling (tile_position (0,0)/(64,0)), bf16, K=64 each
  exp   one ACT pass per m-chunk drains both heads' scores PSUM -> SBUF
        bf16 with the 1/sqrt(DH) scale folded in
  PV    [V|1]^T @ expS.T -> O'.T rows 0:64 + softmax rowsum in row 64
  divide DVE reciprocal + PE ones-matmul partition-broadcast + DVE muls
        (h1's mul writes partition-shifted)
  proj  O.T^T @ wpT (fp32r, full-rate) -> fp16 partial, one DMA per block;
        each block's projection is deferred into the next block's stream
"""

import os
import sys

import numpy as np

for _p in ("/opt/trn_rl_repo",):
    if os.path.isdir(_p) and _p not in sys.path:
        sys.path.insert(0, _p)

N, D, H = 2048, 1024, 16
DH = D // H                 # 64
NCORES = 8
HPC = H // NCORES           # 2 heads per core
P = 128
SCALE = 1.0 / DH ** 0.5

D_CHUNKS = D // P           # 8

# config knobs (overridable before run() for experiments)
USE_ROW_TILING = os.environ.get("ATTN_ROW_TILING", "1") == "1"
QKV_DTYPE = os.environ.get("ATTN_QKV_DTYPE", "bfloat16")   # float32r|float32|bfloat16
PROJ_DTYPE = os.environ.get("ATTN_PROJ_DTYPE", "float32r")
ES_BUFS = int(os.environ.get("ATTN_ES_BUFS", "6"))
SPS_BUFS = int(os.environ.get("ATTN_SPS_BUFS", "2"))
NB = int(os.environ.get("ATTN_NB", "512"))                 # query-block size
PV_FP8 = os.environ.get("ATTN_PV_FP8", "0") == "1"         # DoubleRow fp8 PV


def _build_nc(n=N, nb=NB):
    """Build the per-core Bass module (SPMD: identical program, per-core data)."""
    import concourse.bass as bass  # noqa: F401
    import concourse.mybir as mybir
    import concourse.tile as tile
    from concourse import bacc
    from concourse.masks import make_identity

    f32 = mybir.dt.float32
    bf16 = mybir.dt.bfloat16
    f32r = mybir.dt.float32r
    AF = mybir.ActivationFunctionType

    m_chunks = n // P
    n_blocks = n // nb

    dtmap = {"float32r": f32r, "float32": f32, "bfloat16": bf16}
    qkv_sb_dt = dtmap[QKV_DTYPE]
    proj_sb_dt = dtmap[PROJ_DTYPE]

    nc = bacc.Bacc(
        "TRN2",
        target_bir_lowering=False,
        debug=False,
        enable_asserts=True,
        num_devices=NCORES,
    )

    xT_d = nc.dram_tensor("xT", (D, n), qkv_sb_dt, kind="ExternalInput")
    wqkvT_d = nc.dram_tensor("wqkvT", (P, 3, D_CHUNKS, P), qkv_sb_dt, kind="ExternalInput")
    wpT_d = nc.dram_tensor("wpT", (P, D), proj_sb_dt, kind="ExternalInput")
    bqkv_d = nc.dram_tensor("bqkv", (P, 3), f32, kind="ExternalInput")
    f16 = mybir.dt.float16
    out_d = nc.dram_tensor("out_part", (n, D), f16, kind="ExternalOutput")

    with tile.TileContext(nc) as tc:
        with (
            tc.tile_pool(name="consts", bufs=1) as consts,
            tc.tile_pool(name="xpool", bufs=1) as xpool,
            tc.tile_pool(name="qkpool", bufs=1) as qkpool,
        ):
            # ---- inputs ----
            # wqkv loads per part (k first: it gates the first scores matmul);
            # x streams in (j, o) pieces so attention can start while x loads.
            wqkv_sb = consts.tile([P, 3, D_CHUNKS, P], qkv_sb_dt)
            wp_sb = consts.tile([P, D], proj_sb_dt)
            bqkv_sb = consts.tile([P, 3], f32)
            xT_sb = xpool.tile([P, D_CHUNKS, n], qkv_sb_dt)

            qw = min(512, n)
            n_j = n // qw
            nc.sync.dma_start(bqkv_sb[:], bqkv_d.ap())
            # part order: k(1), q(0), v(2); host sends wqkvT part-major so
            # each part's weight DMA is one contiguous 4KB run per partition
            PART_ORDER = (1, 0, 2)
            nc.sync.dma_start(wqkv_sb[:, 1], wqkvT_d.ap()[:, 1])

            def x_piece(j0, j1, o):
                nc.sync.dma_start(
                    xT_sb[:, o, j0 * qw:j1 * qw],
                    xT_d.ap()[o * P:(o + 1) * P, j0 * qw:j1 * qw],
                )

            for o in range(D_CHUNKS):
                x_piece(0, 1, o)
            nc.sync.dma_start(wqkv_sb[:, 0], wqkvT_d.ap()[:, 0])
            nc.sync.dma_start(wqkv_sb[:, 2], wqkvT_d.ap()[:, 2])
            for j in range(1, n_j):
                for o in range(D_CHUNKS):
                    x_piece(j, j + 1, o)
            nc.sync.dma_start(wp_sb[:], wpT_d.ap())

            ident = consts.tile([P, P], bf16)
            make_identity(nc, ident[:])
            # ones row at partition DH (aligned with PV rowsum row) for the
            # reciprocal partition-broadcast matmul (f32r for 1 cyc/row)
            ones_f32 = consts.tile([P, DH], f32)
            nc.gpsimd.memset(ones_f32[0:1, :], 1.0)
            ones_sb = consts.tile([P, DH], f32r)
            nc.vector.tensor_copy(ones_sb[0:1, :], ones_f32[0:1, :])

            # ---- persistent activations ----
            qT_sb = qkpool.tile([P, n], bf16)           # Q.T (head-dim on parts)
            kT_sb = qkpool.tile([P, n], bf16)           # K.T
            vT_sb = qkpool.tile([P, n], bf16)           # V.T (pre-transpose)
            fp8 = mybir.dt.float8e4
            if PV_FP8:
                # [pair, i, cols]: h0 at 0:65 (V|1), h1 at 80:145 (V|1);
                # row stride 160 and h offsets are 16B-aligned for DoubleRow
                v_sb = qkpool.tile([P, m_chunks // 2, 2, 160], fp8)
                nc.gpsimd.memset(v_sb[:, :, :, DH:DH + 1], 1.0)
                nc.gpsimd.memset(v_sb[:, :, :, 80 + DH:80 + DH + 1], 1.0)
            else:
                v_sb = qkpool.tile([P, m_chunks, 2 * (DH + 1)], bf16)  # [V_h0|1|V_h1|1]
                nc.gpsimd.memset(v_sb[:, :, DH:DH + 1], 1.0)
                nc.gpsimd.memset(v_sb[:, :, 2 * DH + 1:2 * DH + 2], 1.0)

            # ===== PSUM pools: one global budget, no phase aliasing =====
            # accp: QKV accumulators + transposes + proj + recip-bcast (1 bank)
            # sps:  scores tiles (2 banks each)
            # pvps: PV accumulators (1 bank each)
            # total: 2*1 + 2*2 + 2*1 = 8 banks
            accp = tc.alloc_tile_pool(name="accp", bufs=2, space="PSUM")
            sps = tc.alloc_tile_pool(name="sps", bufs=SPS_BUFS, space="PSUM")
            pvps = tc.alloc_tile_pool(name="pvps", bufs=2, space="PSUM")

            # ================= Phases: QKV + attention, interleaved ==========
            # Emission order drives Tile priorities:
            #   j0:(k,q,v)+transposes -> attn(b0, mc group j0) -> j1:(...) ...
            # then blocks 1..; each block's projection is deferred into the
            # next block's stream so it fills PE gaps instead of stalling ACT.
            dst = {0: qT_sb, 1: kT_sb, 2: vT_sb}

            def qkv_j(j, parts=PART_ORDER):
                for part in parts:
                    ps = accp.tile([P, qw], mybir.dt.float32, tag="acc",
                                   name=f"qkv_ps_{part}_{j}")
                    for o in range(D_CHUNKS):
                        nc.tensor.matmul(
                            ps[:],
                            wqkv_sb[:, part, o, :],
                            xT_sb[:, o, j * qw:(j + 1) * qw],
                            start=(o == 0),
                            stop=(o == D_CHUNKS - 1),
                        )
                    # drain with bias add (per-partition scalar), cast bf16
                    nc.vector.tensor_add(
                        dst[part][:, j * qw:(j + 1) * qw],
                        ps[:],
                        bqkv_sb[:, part:part + 1].broadcast_to([P, qw]),
                    )
                    if part == 2:
                        # V.T -> V for the m-chunks covered by this j slice
                        for mc in range(j * qw // P, (j + 1) * qw // P):
                            tp = accp.tile([P, P], bf16, tag="acc", name=f"tp_{mc}")
                            nc.tensor.transpose(
                                tp[:], vT_sb[:, mc * P:(mc + 1) * P], ident[:]
                            )
                            if PV_FP8:
                                g, i = mc // 2, mc % 2
                                nc.vector.tensor_copy(
                                    v_sb[:, g, i, 0:DH], tp[:, 0:DH]
                                )
                                nc.vector.tensor_copy(
                                    v_sb[:, g, i, 80:80 + DH], tp[:, DH:2 * DH]
                                )
                            else:
                                nc.vector.tensor_copy(v_sb[:, mc, 0:DH], tp[:, 0:DH])
                                nc.vector.tensor_copy(
                                    v_sb[:, mc, DH + 1:2 * DH + 1], tp[:, DH:2 * DH]
                                )

            # variable-size query blocks; small final block shrinks the tail
            bw = min(nb, n)
            blocks = [bw] * (n // bw)

            with (
                tc.tile_pool(name="espool", bufs=ES_BUFS) as espool,
                tc.tile_pool(name="opool", bufs=2) as opool,
                tc.tile_pool(name="outpool", bufs=2) as outpool,
                tc.tile_pool(name="rpool", bufs=2) as rpool,
            ):
                pps = accp

                def attn_sc(b, row0, nbb, mcs):
                    nsl = slice(row0, row0 + nbb)
                    out = []
                    es_pair = None
                    for mc in mcs:
                        s_ps = sps.tile([P, 2 * nbb], mybir.dt.float32, tag="s",
                                        name=f"s_ps_{b}_{mc}")
                        for h in range(HPC):
                            nc.tensor.matmul(
                                s_ps[:, h * nbb:(h + 1) * nbb],
                                kT_sb[h * DH:(h + 1) * DH, mc * P:(mc + 1) * P],
                                qT_sb[h * DH:(h + 1) * DH, nsl],
                                tile_position=(h * DH, 0) if USE_ROW_TILING else None,
                            )
                        if PV_FP8:
                            if mc % 2 == 0:
                                es_pair = espool.tile([P, 2, 2 * nbb], fp8, tag="es",
                                                      name=f"es_{b}_{mc}")
                            nc.scalar.activation(es_pair[:, mc % 2, :], s_ps[:],
                                                 AF.Exp, scale=SCALE)
                            if mc % 2 == 1:
                                out.append((mc // 2, es_pair))
                        else:
                            es = espool.tile([P, 2 * nbb], bf16, tag="es",
                                             name=f"es_{b}_{mc}")
                            nc.scalar.activation(es[:], s_ps[:], AF.Exp, scale=SCALE)
                            out.append((mc, es))
                    return out

                def attn_pv(nbb, pvs, mc_es):
                    if PV_FP8:
                        for g, es_pair in mc_es:
                            for h in range(HPC):
                                nc.tensor.matmul(
                                    pvs[h][0:DH + 1, :],
                                    v_sb[:, g, :, 80 * h:80 * h + DH + 1],
                                    es_pair[:, :, h * nbb:(h + 1) * nbb],
                                    start=(g == 0),
                                    stop=(g == m_chunks // 2 - 1),
                                    perf_mode=mybir.MatmulPerfMode.DoubleRow,
                                )
                        return
                    for mc, es in mc_es:
                        for h in range(HPC):
                            nc.tensor.matmul(
                                pvs[h][0:DH + 1, :],
                                v_sb[:, mc, h * (DH + 1):(h + 1) * (DH + 1)],
                                es[:, h * nbb:(h + 1) * nbb],
                                start=(mc == 0),
                                stop=(mc == m_chunks - 1),
                            )

                def attn_mc_group(b, row0, nbb, pvs, mcs):
                    attn_pv(nbb, pvs, attn_sc(b, row0, nbb, mcs))

                def division(b, nbb, pvs):
                    # O.T = O'.T / rowsum, heads stacked on partitions.
                    # h1's mul writes partition-shifted (verified on HW).
                    rt = rpool.tile([P, HPC * nbb], mybir.dt.float32r, tag="recip",
                                    name=f"rt_{b}")
                    rb = rpool.tile([P, HPC * nbb], mybir.dt.float32, tag="rbcast",
                                    name=f"rb_{b}")
                    oT = opool.tile([P, nbb], proj_sb_dt, tag="oT", name=f"oT_{b}")
                    for h in range(HPC):
                        hs = slice(h * nbb, (h + 1) * nbb)
                        with nc.allow_low_precision(reason="f32r recip, bcast mm"):
                            nc.vector.reciprocal(rt[0:1, hs], pvs[h][DH:DH + 1, :])
                        rb_ps = pps.tile([P, nbb], mybir.dt.float32, tag="acc",
                                         name=f"rb_ps_{b}_{h}")
                        nc.tensor.matmul(rb_ps[0:DH, :], ones_sb[0:1, :], rt[0:1, hs])
                        nc.vector.tensor_copy(rb[0:DH, hs], rb_ps[0:DH, :])
                        nc.vector.tensor_mul(
                            oT[h * DH:(h + 1) * DH, :],
                            pvs[h][0:DH, :],
                            rb[0:DH, hs],
                        )
                    return oT

                def projection(b, row0, nbb, oT, last=False):
                    nch = nbb // P
                    out_sb = outpool.tile([P, nch, D], f16, tag="out",
                                          name=f"out_{b}")
                    for j in range(nch):
                        for half in range(D // 512):
                            pp = pps.tile([P, 512], mybir.dt.float32, tag="acc",
                                          name=f"pp_{b}_{j}_{half}")
                            nc.tensor.matmul(
                                pp[:],
                                oT[:, j * P:(j + 1) * P],
                                wp_sb[:, half * 512:(half + 1) * 512],
                            )
                            dslc = out_sb[:, j, half * 512:(half + 1) * 512]
                            if last and (j % 2 == 1):
                                # ACT is idle in the tail; split drains across
                                # both engines to shorten the epilogue chain
                                nc.scalar.copy(dslc, pp[:])
                            else:
                                nc.vector.tensor_copy(dslc, pp[:])
                    nc.sync.dma_start(
                        out_d.ap()[row0:row0 + nbb, :].rearrange(
                            "(c p) d -> p c d", p=P
                        ),
                        out_sb[:],
                    )

                pending = None   # (b, row0, nbb, oT) awaiting projection
                row0 = 0
                for b, nbb in enumerate(blocks):
                    pvs = [
                        pvps.tile([P, nbb], mybir.dt.float32, tag="pv",
                                  name=f"pv_{b}_{h}")
                        for h in range(HPC)
                    ]
                    if b == 0:
                        # fine interleave with QKV j-sweeps: k-slice -> scores
                        # -> v-slice(+transpose) -> PV; q for j0 plus block 1's
                        # q-slice at the end (each later block's q-slice is
                        # pre-emitted inside the previous block's stream so it
                        # never sits on the block-boundary critical path)
                        mcs_per_j = qw // P
                        for j in range(n_j):
                            qkv_j(j, parts=(1, 0) if j == 0 else (1,))
                            mc_es = attn_sc(b, row0, nbb,
                                            range(j * mcs_per_j, (j + 1) * mcs_per_j))
                            qkv_j(j, parts=(2,))
                            attn_pv(nbb, pvs, mc_es)
                        if n_j > 1:
                            qkv_j(1, parts=(0,))
                    else:
                        # deferred projection + next block's q-slice interleave
                        # after the first mcs (PE slack while ACT streams exps)
                        split = min(4, m_chunks)
                        attn_mc_group(b, row0, nbb, pvs, range(0, split))
                        if pending is not None:
                            projection(*pending)
                            pending = None
                        if b + 1 < n_j:
                            qkv_j(b + 1, parts=(0,))
                        attn_mc_group(b, row0, nbb, pvs, range(split, m_chunks))
                    oT = division(b, nbb, pvs)
                    if pending is not None:
                        projection(*pending)
                        pending = None
                    pending = (b, row0, nbb, oT)
                    row0 += nbb
                projection(*pending, last=True)

            pvps.release()
            sps.release()
            accp.release()

    nc.compile()
    return nc


def _host_prep(x, w_qkv, b_qkv, w_proj, n=N):
    """Per-core input maps (dtypes match the DRAM tensor declarations)."""
    import ml_dtypes

    qkv_np = (ml_dtypes.bfloat16 if QKV_DTYPE == "bfloat16" else np.float32)
    proj_np = (ml_dtypes.bfloat16 if PROJ_DTYPE == "bfloat16" else np.float32)
    xT = np.ascontiguousarray(x.T.astype(qkv_np))
    in_maps = []
    for c in range(NCORES):
        wq = w_qkv[0 * D + c * P:0 * D + (c + 1) * P, :]
        wk = w_qkv[1 * D + c * P:1 * D + (c + 1) * P, :]
        wv = w_qkv[2 * D + c * P:2 * D + (c + 1) * P, :]
        # part-major [p, part, o, c]: contiguous per-part weight DMAs
        wqkvT = np.ascontiguousarray(
            np.stack(
                [a.T.reshape(D_CHUNKS, P, P).transpose(1, 0, 2) for a in (wq, wk, wv)],
                axis=1,
            ).astype(qkv_np)
        )
        wpT = np.ascontiguousarray(w_proj[:, c * P:(c + 1) * P].T.astype(proj_np))
        bq = b_qkv[0 * D + c * P:0 * D + (c + 1) * P]
        bk = b_qkv[1 * D + c * P:1 * D + (c + 1) * P]
        bv = b_qkv[2 * D + c * P:2 * D + (c + 1) * P]
        bqkv = np.ascontiguousarray(
            np.stack([bq, bk, bv], axis=1).astype(np.float32)
        )
        in_maps.append({"xT": xT, "wqkvT": wqkvT, "wpT": wpT, "bqkv": bqkv})
    return in_maps


_NC_CACHE = {}


def run(x, w_qkv, b_qkv, w_proj, b_proj, trace=False, n=N, nb=None, **spmd_kwargs):
    from concourse.bass_utils import run_bass_kernel_spmd

    if nb is None:
        nb = NB
    key = (n, nb, USE_ROW_TILING, QKV_DTYPE, PROJ_DTYPE, ES_BUFS, SPS_BUFS, PV_FP8)
    if key not in _NC_CACHE:
        _NC_CACHE[key] = _build_nc(n=n, nb=nb)
    nc = _NC_CACHE[key]

    in_maps = _host_prep(
        np.asarray(x), np.asarray(w_qkv), np.asarray(b_qkv), np.asarray(w_proj), n=n
    )
    results = run_bass_kernel_spmd(
        nc, in_maps, core_ids=list(range(NCORES)), trace=trace, **spmd_kwargs
    )
    acc = np.zeros((n, D), dtype=np.float64)
    for c in range(NCORES):
        acc += results.results[c]["out_part"].astype(np.float64)
    acc += np.asarray(b_proj).astype(np.float64)
    return acc.astype(np.float32), results


def kernel(x, w_qkv, b_qkv, w_proj, b_proj):
    out, _ = run(x, w_qkv, b_qkv, w_proj, b_proj, trace=False)
    return out



# revision 14
# speedup vs baseline: 1.2445x; 1.2445x over previous
"""Multi-head attention (N=2048, D=1024, H=16) on 8 TRN2 NeuronCores.

Sharding: tensor-parallel over heads (2 heads / core). x is replicated
(pre-transposed + pre-cast on host), each core computes QKV / scores /
softmax / PV / out-proj for its 2 heads, producing a partial (N, D)
projection output in fp16. The all-reduce over cores is the host-side
f64 sum of the 8 partials (+ b_proj), cast back to f32.

Device inputs (per core):
  xT      (D, N)    bf16 : x transposed (host prep)
  wqkvT   (128,3,8,128) bf16 : [Wq.T | Wk.T | Wv.T] slices, part-major
  wpT     (128, D)  bf16 : w_proj[:, core_cols].T
  bqkv    (128, 3)  f32  : [bq | bk | bv] slices
  out_part(N, D)    f16  : partial projection output

Per-core pipeline v3 (ACT-bound design):
  QKV   bf16 matmuls j-sliced; K drains feed block-0 scores progressively.
        Q/K drain to fp8e4 with a zero second "DoubleRow member";
        V drains bf16 then PE-transposes to seq-major [V_h0|1|V_h1|1].
  scores fp8e4 DoubleRow (2x PE rate): lhsT = K.T pair-slice, rhs = Q.T
        pair-slice; the zero member contributes nothing.
  exp   one ACT pass per m-chunk (the kernel bottleneck: 64 x 1024 elems)
        into a per-block es array (ping-pong, bufs=2).
  PV    deferred one block (es array complete): per (qc, head) stream,
        lhsT = es chunk (128 keys x 128 q), rhs = [V|1] (128 keys x 65),
        16 back-to-back accumulating matmuls into a single-stream PSUM
        bank (PSUM start zeroes whole banks here - one stream per bank).
  div   rowsum is column 64 (q-major): DVE reciprocal + per-partition
        broadcast mul -> O bf16. PE transpose -> O.T.
  proj  O.T^T @ wpT in bf16 -> fp16 partial, one DMA per 128-row chunk.
  All of PV/div/proj for block b rides inside block b+1's scores/exp
  stream so the ACT exp pipeline never waits on PE.
"""

import os
import sys

import numpy as np

for _p in ("/opt/trn_rl_repo",):
    if os.path.isdir(_p) and _p not in sys.path:
        sys.path.insert(0, _p)

N, D, H = 2048, 1024, 16
DH = D // H                 # 64
NCORES = 8
HPC = H // NCORES           # 2 heads per core
P = 128
SCALE = 1.0 / DH ** 0.5

D_CHUNKS = D // P           # 8

# config knobs (overridable before run() for experiments)
SCORES_FP8 = os.environ.get("ATTN_SCORES_FP8", "0") == "1"
NB = int(os.environ.get("ATTN_NB", "512"))                 # query-block size
WARMUP = int(os.environ.get("ATTN_WARMUP", "24"))          # PE warmup transposes
ES_BUFS = int(os.environ.get("ATTN_ES_BUFS", "3"))         # es block arrays


def _build_nc(n=N, nb=NB):
    """Build the per-core Bass module (SPMD: identical program, per-core data)."""
    import concourse.bass as bass  # noqa: F401
    import concourse.mybir as mybir
    import concourse.tile as tile
    from concourse import bacc
    from concourse.masks import make_identity

    f32 = mybir.dt.float32
    bf16 = mybir.dt.bfloat16
    f16 = mybir.dt.float16
    fp8 = mybir.dt.float8e4
    AF = mybir.ActivationFunctionType
    DR = mybir.MatmulPerfMode.DoubleRow

    m_chunks = n // P           # 16
    n_blocks = n // nb          # 4
    QC = nb // P                # 4 query chunks per block

    nc = bacc.Bacc(
        "TRN2",
        target_bir_lowering=False,
        debug=False,
        enable_asserts=True,
        num_devices=NCORES,
    )

    xT_d = nc.dram_tensor("xT", (D, n), bf16, kind="ExternalInput")
    wqkvT_d = nc.dram_tensor("wqkvT", (P, 3, D_CHUNKS, P), bf16, kind="ExternalInput")
    wpT_d = nc.dram_tensor("wpT", (P, D), bf16, kind="ExternalInput")
    bqkv_d = nc.dram_tensor("bqkv", (P, 3), f32, kind="ExternalInput")
    out_d = nc.dram_tensor("out_part", (n, D), f16, kind="ExternalOutput")

    with tile.TileContext(nc) as tc:
        with (
            tc.tile_pool(name="consts", bufs=1) as consts,
            tc.tile_pool(name="xpool", bufs=1) as xpool,
            tc.tile_pool(name="qkpool", bufs=1) as qkpool,
        ):
            # ---- inputs ----
            wqkv_sb = consts.tile([P, 3, D_CHUNKS, P], bf16)
            wp_sb = consts.tile([P, D], bf16)
            bqkv_sb = consts.tile([P, 3], f32)
            xT_sb = xpool.tile([P, D_CHUNKS, n], bf16)

            qw = min(512, n)
            n_j = n // qw
            # part order: k(1), q(0), v(2); one DMA per j-slice of x, except
            # j0 which is split in two o-halves so QKV can start accumulating
            # while the second half streams in
            nc.sync.dma_start(wqkv_sb[:, 1], wqkvT_d.ap()[:, 1])
            nc.sync.dma_start(wqkv_sb[:, 0], wqkvT_d.ap()[:, 0])
            nc.sync.dma_start(bqkv_sb[:], bqkv_d.ap())

            xT_re = xT_d.ap().rearrange("(o p) x -> p o x", p=P)

            def x_slice(j, o0=0, o1=D_CHUNKS):
                nc.sync.dma_start(
                    xT_sb[:, o0:o1, j * qw:(j + 1) * qw],
                    xT_re[:, o0:o1, j * qw:(j + 1) * qw],
                )

            if os.environ.get("ATTN_SPLIT_X0", "1") == "1":
                x_slice(0, 0, 4)
                x_slice(0, 4, 8)
            else:
                x_slice(0)
            nc.sync.dma_start(wqkv_sb[:, 2], wqkvT_d.ap()[:, 2])
            for j in range(1, n_j):
                x_slice(j)
            nc.sync.dma_start(wp_sb[:], wpT_d.ap())

            ident = consts.tile([P, P], bf16)
            make_identity(nc, ident[:])

            # ---- persistent activations ----
            if SCORES_FP8:
                # [p, member, col]: member 1 is an all-zero DoubleRow partner
                qT_sb = qkpool.tile([P, 2, n], fp8)
                kT_sb = qkpool.tile([P, 2, n], fp8)
                nc.gpsimd.memset(qT_sb[:, 1, 0:qw], 0.0)
                nc.gpsimd.memset(kT_sb[:, 1, 0:qw], 0.0)
                nc.gpsimd.memset(qT_sb[:, 1, qw:n], 0.0)
                nc.gpsimd.memset(kT_sb[:, 1, qw:n], 0.0)
            else:
                qT_sb = qkpool.tile([P, 1, n], bf16)
                kT_sb = qkpool.tile([P, 1, n], bf16)
            vT_sb = qkpool.tile([P, n], bf16)
            # [V_h0|1|V_h1|1] seq-major, per m-chunk
            v_sb = qkpool.tile([P, m_chunks, 2 * (DH + 1)], bf16)
            nc.gpsimd.memset(v_sb[:, :, DH:DH + 1], 1.0)
            nc.gpsimd.memset(v_sb[:, :, 2 * DH + 1:2 * DH + 2], 1.0)
            # es arrays: one full block of exp(S) per buffer; 3 bufs so the
            # exp stream of block b never waits on PV of block b-ES_BUFS+1
            es_arr = [
                qkpool.tile([P, m_chunks, 2 * nb], bf16, name=f"es_arr{i}")
                for i in range(ES_BUFS)
            ]

            # ===== PSUM pools (8 banks): sps 2x2 + pvps 2x1 + accp 2x1 =====
            accp = tc.alloc_tile_pool(name="accp", bufs=2, space="PSUM")
            sps = tc.alloc_tile_pool(name="sps", bufs=2, space="PSUM")
            pvps = tc.alloc_tile_pool(name="pvps", bufs=2, space="PSUM")

            for wi in range(WARMUP):
                wt = accp.tile([P, P], bf16, tag="acc", name=f"warm_{wi}")
                nc.tensor.transpose(wt[:], ident[:], ident[:])

            dst = {0: qT_sb, 1: kT_sb}

            def qkv_j(j, parts, two_phase=False):
                for part in parts:
                    ps = accp.tile([P, qw], f32, tag="acc",
                                   name=f"qkv_ps_{part}_{j}")
                    if two_phase:
                        # accumulate o 0:4 (first x half-DMA) then 4:8 so
                        # the PE starts before the full slice has landed
                        for o in range(4):
                            nc.tensor.matmul(
                                ps[:], wqkv_sb[:, part, o, :],
                                xT_sb[:, o, j * qw:(j + 1) * qw],
                                start=(o == 0), stop=False,
                            )
                        for o in range(4, D_CHUNKS):
                            nc.tensor.matmul(
                                ps[:], wqkv_sb[:, part, o, :],
                                xT_sb[:, o, j * qw:(j + 1) * qw],
                                start=False, stop=(o == D_CHUNKS - 1),
                            )
                    else:
                        for o in range(D_CHUNKS):
                            nc.tensor.matmul(
                                ps[:], wqkv_sb[:, part, o, :],
                                xT_sb[:, o, j * qw:(j + 1) * qw],
                                start=(o == 0), stop=(o == D_CHUNKS - 1),
                            )
                    jsl = slice(j * qw, (j + 1) * qw)
                    if part == 2:
                        nc.vector.tensor_add(
                            vT_sb[:, jsl], ps[:],
                            bqkv_sb[:, 2:3].broadcast_to([P, qw]),
                        )
                        # V.T -> V (seq-major) for this j's m-chunks, batched
                        mc0 = j * qw // P
                        nmc = qw // P
                        tp = accp.tile([P, nmc, P], bf16, tag="acc",
                                       name=f"tp_{j}")
                        for i in range(nmc):
                            nc.tensor.transpose(
                                tp[:, i, :],
                                vT_sb[:, (mc0 + i) * P:(mc0 + i + 1) * P],
                                ident[:],
                            )
                        nc.vector.tensor_copy(
                            v_sb[:, mc0:mc0 + nmc, 0:DH], tp[:, :, 0:DH]
                        )
                        nc.vector.tensor_copy(
                            v_sb[:, mc0:mc0 + nmc, DH + 1:2 * DH + 1],
                            tp[:, :, DH:2 * DH],
                        )
                    else:
                        with nc.allow_low_precision(reason="qk fp8 drain"):
                            nc.vector.tensor_add(
                                dst[part][:, 0, jsl], ps[:],
                                bqkv_sb[:, part:part + 1].broadcast_to([P, qw]),
                            )

            with (
                tc.tile_pool(name="opool", bufs=2) as opool,
                tc.tile_pool(name="otpool", bufs=2) as otpool,
                tc.tile_pool(name="outpool", bufs=2) as outpool,
                tc.tile_pool(name="zrpool", bufs=4) as zrpool,
            ):
                def scores_exp(b, row0, nbb, mc):
                    """Scores + exp for one m-chunk -> es_arr[b%ES][:, mc]."""
                    nsl = slice(row0, row0 + nbb)
                    s_ps = sps.tile([P, 2 * nbb], f32, tag="s",
                                    name=f"s_ps_{b}_{mc}")
                    for h in range(HPC):
                        hsl = slice(h * DH, (h + 1) * DH)
                        if SCORES_FP8:
                            nc.tensor.matmul(
                                s_ps[:, h * nbb:(h + 1) * nbb],
                                kT_sb[hsl, :, mc * P:(mc + 1) * P],
                                qT_sb[hsl, :, nsl],
                                perf_mode=DR,
                            )
                        else:
                            nc.tensor.matmul(
                                s_ps[:, h * nbb:(h + 1) * nbb],
                                kT_sb[hsl, 0, mc * P:(mc + 1) * P],
                                qT_sb[hsl, 0, nsl],
                            )
                    nc.scalar.activation(es_arr[b % ES_BUFS][:, mc, :], s_ps[:],
                                         AF.Exp, scale=SCALE)

                def pv_stream(b, nbb, qc, h):
                    """One PV accumulation stream (own PSUM bank)."""
                    es = es_arr[b % ES_BUFS]
                    pv = pvps.tile([P, DH + 1], f32, tag="pv",
                                   name=f"pv_{b}_{qc}_{h}")
                    for mc in range(m_chunks):
                        nc.tensor.matmul(
                            pv[:],
                            es[:, mc, h * nbb + qc * P:h * nbb + (qc + 1) * P],
                            v_sb[:, mc, h * (DH + 1):(h + 1) * (DH + 1)],
                            start=(mc == 0),
                            stop=(mc == m_chunks - 1),
                        )
                    return pv

                def division(b, qc, h, pv, ov):
                    """DVE: O[:, h] = O'/rowsum (q-major)."""
                    zr = zrpool.tile([P, 1], f32, tag="zr",
                                     name=f"zr_{b}_{qc}_{h}")
                    with nc.allow_low_precision(reason="softmax recip"):
                        nc.vector.reciprocal(zr[:], pv[:, DH:DH + 1])
                    nc.vector.tensor_mul(
                        ov[:, h, :], pv[:, 0:DH],
                        zr[:].broadcast_to([P, DH]),
                    )

                def pv_div_qc(b, nbb, qc):
                    """Both heads' PV streams + divisions for one q-chunk."""
                    ov = opool.tile([P, HPC, DH], bf16, tag="o",
                                    name=f"o_{b}_{qc}")
                    for h in range(HPC):
                        pv = pv_stream(b, nbb, qc, h)
                        division(b, qc, h, pv, ov)
                    return ov

                def project_qc(b, row0, qc, ov, last=False):
                    """PE transpose + projection + drains + out DMA."""
                    oT_ps = accp.tile([P, P], bf16, tag="acc",
                                      name=f"oT_ps_{b}_{qc}")
                    nc.tensor.transpose(oT_ps[:], ov[:], ident[:])
                    oT = otpool.tile([P, P], bf16, tag="oT",
                                     name=f"oT_{b}_{qc}")
                    if last and qc >= 2:
                        nc.scalar.copy(oT[:], oT_ps[:])
                    else:
                        nc.vector.tensor_copy(oT[:], oT_ps[:])
                    out_sb = outpool.tile([P, D], f16, tag="out",
                                          name=f"out_{b}_{qc}")
                    for half in range(2):
                        pp = accp.tile([P, 512], f32, tag="acc",
                                       name=f"pp_{b}_{qc}_{half}")
                        nc.tensor.matmul(
                            pp[:], oT[:],
                            wp_sb[:, half * 512:(half + 1) * 512],
                        )
                        dslc = out_sb[:, half * 512:(half + 1) * 512]
                        if last and half == 1:
                            nc.scalar.copy(dslc, pp[:])
                        else:
                            nc.vector.tensor_copy(dslc, pp[:])
                    row = row0 + qc * P
                    nc.sync.dma_start(out_d.ap()[row:row + P, :], out_sb[:])

                # ================= main schedule =================
                assert nb == qw, "block size must match j-slice width"
                blocks = [nb] * n_blocks
                mcs_per_j = qw // P

                # PV/div/proj work queue: entries (b, row0, qc), popped into
                # later blocks' PE slack once all of v_sb has been emitted.
                from collections import deque
                pvq = deque()
                # pops per g for blocks 1..: v(j1..3) occupy b1's early slack
                pop_budget = {1: [0, 0, 1, 1], 2: [1, 1, 2, 2], 3: [1, 1, 1, 1]}

                def pop_pv(k, last=False):
                    for _ in range(k):
                        if not pvq:
                            return
                        pb, prow0, qc = pvq.popleft()
                        ov = pv_div_qc(pb, nb, qc)
                        project_qc(pb, prow0, qc, ov, last=last)

                row0 = 0
                for b, nbb in enumerate(blocks):
                    if b == 0:
                        # interleave with QKV j-sweeps: k(j) gates scores of
                        # its m-chunks; q(j0) first (2-phase, rides the split
                        # x DMA); v(j1..j3) deferred into block 1 to keep
                        # block 0's PE load under the exp window.
                        qkv_j(0, parts=(0, 1),
                              two_phase=os.environ.get("ATTN_SPLIT_X0", "1") == "1")
                        for j in range(n_j):
                            if j > 0:
                                qkv_j(j, parts=(1,))
                            for mc in range(j * mcs_per_j, (j + 1) * mcs_per_j):
                                scores_exp(b, row0, nbb, mc)
                            if j == 0:
                                qkv_j(0, parts=(2,))
                            if j == 1:
                                qkv_j(1, parts=(0,))
                    else:
                        # block b's scores/exp feed ACT; queued PV/div/proj
                        # (and leftover QKV v-parts) ride the PE slack
                        for g in range(4):
                            for mc in range(g * 4, g * 4 + 4):
                                scores_exp(b, row0, nbb, mc)
                            if b == 1 and g < 3:
                                qkv_j(g + 1, parts=(2,))
                            if g == 0 and b + 1 < n_blocks:
                                qkv_j(b + 1, parts=(0,))
                            pop_pv(pop_budget[b][g])
                    for qc in range(QC):
                        pvq.append((b, row0, qc))
                    row0 += nbb
                # epilogue: drain the remaining queue (the last block's qc's)
                pop_pv(len(pvq), last=True)

            pvps.release()
            sps.release()
            accp.release()

    nc.compile()
    return nc


def _host_prep(x, w_qkv, b_qkv, w_proj, n=N):
    """Per-core input maps (dtypes match the DRAM tensor declarations)."""
    import ml_dtypes

    bf = ml_dtypes.bfloat16
    xT = np.ascontiguousarray(x.T.astype(bf))
    in_maps = []
    for c in range(NCORES):
        wq = w_qkv[0 * D + c * P:0 * D + (c + 1) * P, :]
        wk = w_qkv[1 * D + c * P:1 * D + (c + 1) * P, :]
        wv = w_qkv[2 * D + c * P:2 * D + (c + 1) * P, :]
        # part-major [p, part, o, c]: contiguous per-part weight DMAs
        wqkvT = np.ascontiguousarray(
            np.stack(
                [a.T.reshape(D_CHUNKS, P, P).transpose(1, 0, 2) for a in (wq, wk, wv)],
                axis=1,
            ).astype(bf)
        )
        wpT = np.ascontiguousarray(w_proj[:, c * P:(c + 1) * P].T.astype(bf))
        bq = b_qkv[0 * D + c * P:0 * D + (c + 1) * P]
        bk = b_qkv[1 * D + c * P:1 * D + (c + 1) * P]
        bv = b_qkv[2 * D + c * P:2 * D + (c + 1) * P]
        bqkv = np.ascontiguousarray(
            np.stack([bq, bk, bv], axis=1).astype(np.float32)
        )
        in_maps.append({"xT": xT, "wqkvT": wqkvT, "wpT": wpT, "bqkv": bqkv})
    return in_maps


_NC_CACHE = {}


def run(x, w_qkv, b_qkv, w_proj, b_proj, trace=False, n=N, nb=None, **spmd_kwargs):
    from concourse.bass_utils import run_bass_kernel_spmd

    if nb is None:
        nb = NB
    key = (n, nb, SCORES_FP8, WARMUP)
    if key not in _NC_CACHE:
        _NC_CACHE[key] = _build_nc(n=n, nb=nb)
    nc = _NC_CACHE[key]

    in_maps = _host_prep(
        np.asarray(x), np.asarray(w_qkv), np.asarray(b_qkv), np.asarray(w_proj), n=n
    )
    results = run_bass_kernel_spmd(
        nc, in_maps, core_ids=list(range(NCORES)), trace=trace, **spmd_kwargs
    )
    acc = np.zeros((n, D), dtype=np.float64)
    for c in range(NCORES):
        acc += results.results[c]["out_part"].astype(np.float64)
    acc += np.asarray(b_proj).astype(np.float64)
    return acc.astype(np.float32), results


def kernel(x, w_qkv, b_qkv, w_proj, b_proj):
    out, _ = run(x, w_qkv, b_qkv, w_proj, b_proj, trace=False)
    return out


# revision 74
# speedup vs baseline: 1.2946x; 1.0403x over previous
"""Multi-head attention (N=2048, D=1024, H=16) on 8 TRN2 NeuronCores.

Sharding: tensor-parallel over heads (2 heads / core). x is replicated
(pre-transposed + pre-cast on host), each core computes QKV / scores /
softmax / PV / out-proj for its 2 heads, producing a partial (N, D)
projection output in fp16. The all-reduce over cores is the host-side
f64 sum of the 8 partials (+ b_proj), cast back to f32.

Device inputs (per core):
  xT      (D, N)    bf16 : x transposed (host prep)
  wqkvT   (128,3,8,128) bf16 : [Wq.T | Wk.T | Wv.T] slices, part-major
  wpT     (128, D)  bf16 : w_proj[:, core_cols].T
  bqkv    (128, 3)  f32  : [bq | bk | bv] slices
  out_part(N, D)    f16  : partial projection output

Per-core pipeline (ACT-bound design; exp on the scalar engine is the
64 x 1024-elem roofline, every other engine hides under it):
  QKV   bf16 matmuls j-sliced; x streams per j-slice (j0 split in two
        o-halves riding a reordered DMA queue), K(j) is always the next
        x-slice off the wire so block-0 scores are never queued behind
        lower-priority PE work. V is computed seq-major directly
        (lhsT = x chunk, rhs = wv chunk) into [V_h0|1|V_h1|1]; its bias
        rides a K=1 ones-row matmul (exact through the softmax mean).
  scores lhsT = K.T slice, rhs = Q.T (bf16; fp8e4-DoubleRow knob exists
        but costs ~2.4e-2 rel err - over the gate).
  exp   one ACT pass per m-chunk into per-block es arrays (bufs=4).
  PV    deferred one-or-more blocks behind exp via a work queue popped
        into later blocks' PE slack: per (qc, h) stream, lhsT = es chunk
        (128 keys x 128 q), rhs = [V|1] (128 keys x 65, ap_size 65).
        PSUM accumulators are pre-memset and all matmuls use
        start=False (PSUM "start" zeroes whole banks on this target),
        so streams share banks without clobbering and without WAR
        chains. The last block accumulates mc-major, riding along its
        own exp stream so only division remains after the last exp.
  div   rowsum is column 64 (q-major): DVE reciprocal + per-partition
        broadcast mul -> O bf16. PE transpose -> O.T.
  proj  O.T^T @ wpT in bf16 -> fp16 partial, one DMA per 128x512 chunk.
        The last block's division/projection chains alternate DVE/ACT
        and are emitted breadth-first to pipeline the tail.
"""

import os
import sys

import numpy as np

for _p in ("/opt/trn_rl_repo",):
    if os.path.isdir(_p) and _p not in sys.path:
        sys.path.insert(0, _p)

N, D, H = 2048, 1024, 16
DH = D // H                 # 64
NCORES = 8
HPC = H // NCORES           # 2 heads per core
P = 128
SCALE = 1.0 / DH ** 0.5

D_CHUNKS = D // P           # 8

# config knobs (overridable before run() for experiments)
SCORES_FP8 = os.environ.get("ATTN_SCORES_FP8", "0") == "1"
NB = int(os.environ.get("ATTN_NB", "512"))                 # query-block size
WARMUP = int(os.environ.get("ATTN_WARMUP", "24"))          # PE warmup transposes
ES_BUFS = int(os.environ.get("ATTN_ES_BUFS", "4"))         # es block arrays


def _build_nc(n=N, nb=NB):
    """Build the per-core Bass module (SPMD: identical program, per-core data)."""
    import concourse.bass as bass  # noqa: F401
    import concourse.mybir as mybir
    import concourse.tile as tile
    from concourse import bacc
    from concourse.masks import make_identity

    f32 = mybir.dt.float32
    bf16 = mybir.dt.bfloat16
    f16 = mybir.dt.float16
    fp8 = mybir.dt.float8e4
    AF = mybir.ActivationFunctionType
    DR = mybir.MatmulPerfMode.DoubleRow

    m_chunks = n // P           # 16
    n_blocks = n // nb          # 4
    QC = nb // P                # 4 query chunks per block

    nc = bacc.Bacc(
        "TRN2",
        target_bir_lowering=False,
        debug=False,
        enable_asserts=True,
        num_devices=NCORES,
    )

    xT_d = nc.dram_tensor("xT", (D, n), bf16, kind="ExternalInput")
    wqkvT_d = nc.dram_tensor("wqkvT", (P, 3, D_CHUNKS, P), bf16, kind="ExternalInput")
    wpT_d = nc.dram_tensor("wpT", (P, D), bf16, kind="ExternalInput")
    bqkv_d = nc.dram_tensor("bqkv", (P, 3), f32, kind="ExternalInput")
    bvrow_d = nc.dram_tensor("bvrow", (1, P), bf16, kind="ExternalInput")
    out_d = nc.dram_tensor("out_part", (n, D), f16, kind="ExternalOutput")

    with tile.TileContext(nc) as tc:
        with (
            tc.tile_pool(name="consts", bufs=1) as consts,
            tc.tile_pool(name="xpool", bufs=1) as xpool,
            tc.tile_pool(name="qkpool", bufs=1) as qkpool,
        ):
            # ---- inputs ----
            wqkv_sb = consts.tile([P, 3, D_CHUNKS, P], bf16)
            wp_sb = consts.tile([P, D], bf16)
            bqkv_sb = consts.tile([P, 3], f32)
            xT_sb = xpool.tile([P, D_CHUNKS, n], bf16)

            qw = min(512, n)
            n_j = n // qw
            # one DMA per j-slice of x, except j0 which is split in two
            # o-halves so q/k start accumulating while half 2 streams in;
            # order: x-h1, wq, wk, x-h2 puts the startup chain on the
            # critical path exactly once
            xT_re = xT_d.ap().rearrange("(o p) x -> p o x", p=P)

            def x_slice(j, o0=0, o1=D_CHUNKS):
                nc.sync.dma_start(
                    xT_sb[:, o0:o1, j * qw:(j + 1) * qw],
                    xT_re[:, o0:o1, j * qw:(j + 1) * qw],
                )

            SPLIT_X0 = os.environ.get("ATTN_SPLIT_X0", "1") == "1"
            if SPLIT_X0:
                # startup chain: q's o0:4 matmuls only need x-h1 + wq-h1,
                # so the weight DMAs are split per o-half and interleaved
                x_slice(0, 0, 4)
                nc.sync.dma_start(wqkv_sb[:, 0, 0:4], wqkvT_d.ap()[:, 0, 0:4])
                x_slice(0, 4, 8)
                nc.sync.dma_start(wqkv_sb[:, 0, 4:8], wqkvT_d.ap()[:, 0, 4:8])
                nc.sync.dma_start(wqkv_sb[:, 1, 0:4], wqkvT_d.ap()[:, 1, 0:4])
                nc.sync.dma_start(wqkv_sb[:, 1, 4:8], wqkvT_d.ap()[:, 1, 4:8])
            else:
                nc.sync.dma_start(wqkv_sb[:, 0], wqkvT_d.ap()[:, 0])
                nc.sync.dma_start(wqkv_sb[:, 1], wqkvT_d.ap()[:, 1])
                x_slice(0)
            nc.sync.dma_start(bqkv_sb[:], bqkv_d.ap())
            nc.sync.dma_start(wqkv_sb[:, 2], wqkvT_d.ap()[:, 2])
            for j in range(1, n_j):
                x_slice(j, 0, 4)
                x_slice(j, 4, 8)
            nc.sync.dma_start(wp_sb[:], wpT_d.ap())

            ident = consts.tile([P, P], bf16)
            make_identity(nc, ident[:])

            # ---- persistent activations ----
            if SCORES_FP8:
                # [p, member, col]: member 1 is an all-zero DoubleRow partner
                qT_sb = qkpool.tile([P, 2, n], fp8)
                kT_sb = qkpool.tile([P, 2, n], fp8)
                nc.gpsimd.memset(qT_sb[:, 1, 0:qw], 0.0)
                nc.gpsimd.memset(kT_sb[:, 1, 0:qw], 0.0)
                nc.gpsimd.memset(qT_sb[:, 1, qw:n], 0.0)
                nc.gpsimd.memset(kT_sb[:, 1, qw:n], 0.0)
            else:
                qT_sb = qkpool.tile([P, 1, n], bf16)
                kT_sb = qkpool.tile([P, 1, n], bf16)
            # [V_h0|1|V_h1|1] seq-major, per m-chunk
            v_sb = qkpool.tile([P, m_chunks, 2 * (DH + 1)], bf16)
            nc.gpsimd.memset(v_sb[:, :, DH:DH + 1], 1.0)
            nc.gpsimd.memset(v_sb[:, :, 2 * DH + 1:2 * DH + 2], 1.0)
            # K=1 ones row + bv row: folds the V bias into the PV average
            # (sum_k p_k (v+bv) = O + bv exactly, rowsum column unaffected)
            ones_row = consts.tile([P, P], bf16)
            nc.gpsimd.memset(ones_row[0:1, :], 1.0)
            bvrow_sb = consts.tile([P, P], bf16)
            nc.sync.dma_start(bvrow_sb[0:1, :], bvrow_d.ap())
            # es arrays: one full block of exp(S) per buffer; 3 bufs so the
            # exp stream of block b never waits on PV of block b-ES_BUFS+1
            es_arr = [
                qkpool.tile([P, m_chunks, 2 * nb], bf16, name=f"es_arr{i}")
                for i in range(ES_BUFS)
            ]

            # ===== PSUM pools (8 banks): sps 2x2 + pvps 2x1 + accp 2x1 =====
            accp = tc.alloc_tile_pool(name="accp", bufs=2, space="PSUM")
            sps = tc.alloc_tile_pool(name="sps", bufs=2, space="PSUM")
            pvps = tc.alloc_tile_pool(name="pvps", bufs=2, space="PSUM")

            for wi in range(WARMUP):
                wt = accp.tile([P, P], bf16, tag="acc", name=f"warm_{wi}")
                nc.tensor.transpose(wt[:], ident[:], ident[:])

            dst = {0: qT_sb, 1: kT_sb}

            SKIP_PARTS = set(
                int(c) for c in os.environ.get("ATTN_SKIP_PARTS", "")
                if c.isdigit()
            )

            def qkv_j(j, parts):
                for part in parts:
                    if part in SKIP_PARTS:
                        continue
                    if part == 2:
                        # V computed seq-major directly: lhsT = x chunk
                        # (stationary), rhs = wv chunk -> out [128 seq, 128
                        # dh]; the K=1 ones-row matmul folds in the V bias.
                        # One PSUM bank holds all 4 m-chunks of the j-slice:
                        # pre-memset + start=False accumulation everywhere
                        # (PSUM "start" zeroes whole banks on this target).
                        nmc = qw // P
                        mc0 = j * qw // P
                        vp = accp.tile([P, nmc, P], f32, tag="acc",
                                       name=f"v_ps_{j}")
                        nc.vector.memset(vp[:], 0.0)
                        for i in range(nmc):
                            msl = slice((mc0 + i) * P, (mc0 + i + 1) * P)
                            for o in range(D_CHUNKS):
                                nc.tensor.matmul(
                                    vp[:, i, :],
                                    xT_sb[:, o, msl],
                                    wqkv_sb[:, 2, o, :],
                                    start=False, stop=False,
                                    skip_group_check=True,
                                )
                            nc.tensor.matmul(
                                vp[:, i, :], ones_row[0:1, :], bvrow_sb[0:1, :],
                                start=False, stop=(i == nmc - 1),
                                skip_group_check=True,
                            )
                        nc.vector.tensor_copy(
                            v_sb[:, mc0:mc0 + nmc, 0:DH], vp[:, :, 0:DH]
                        )
                        nc.vector.tensor_copy(
                            v_sb[:, mc0:mc0 + nmc, DH + 1:2 * DH + 1],
                            vp[:, :, DH:2 * DH],
                        )
                        continue
                    ps = accp.tile([P, qw], f32, tag="acc",
                                   name=f"qkv_ps_{part}_{j}")
                    for o in range(D_CHUNKS):
                        nc.tensor.matmul(
                            ps[:], wqkv_sb[:, part, o, :],
                            xT_sb[:, o, j * qw:(j + 1) * qw],
                            start=(o == 0), stop=(o == D_CHUNKS - 1),
                        )
                    with nc.allow_low_precision(reason="qk fp8 drain"):
                        if part == 1:
                            # split the K drain so the slice's first m-chunks
                            # unblock their scores before the full drain
                            for c0, c1 in ((0, P), (P, qw)):
                                nc.vector.tensor_add(
                                    dst[part][:, 0, j * qw + c0:j * qw + c1],
                                    ps[:, c0:c1],
                                    bqkv_sb[:, part:part + 1]
                                    .broadcast_to([P, c1 - c0]),
                                )
                        else:
                            jsl = slice(j * qw, (j + 1) * qw)
                            nc.vector.tensor_add(
                                dst[part][:, 0, jsl], ps[:],
                                bqkv_sb[:, part:part + 1].broadcast_to([P, qw]),
                            )

            def qkv_j0_qk():
                """q+k for j0, interleaved two-phase over the split x DMA;
                k's drain is split so scores(mc0) starts on the first piece."""
                pss = {}
                for part in (0, 1):
                    pss[part] = accp.tile([P, qw], f32, tag="acc",
                                          name=f"qkv_ps_{part}_0")
                for part in (0, 1):
                    if part in SKIP_PARTS:
                        continue
                    for o in range(D_CHUNKS):
                        nc.tensor.matmul(
                            pss[part][:], wqkv_sb[:, part, o, :],
                            xT_sb[:, o, 0:qw],
                            start=(o == 0), stop=(o == D_CHUNKS - 1),
                        )
                with nc.allow_low_precision(reason="qk fp8 drain"):
                    if 0 not in SKIP_PARTS:
                        nc.vector.tensor_add(
                            qT_sb[:, 0, 0:qw], pss[0][:],
                            bqkv_sb[:, 0:1].broadcast_to([P, qw]),
                        )
                    if 1 not in SKIP_PARTS:
                        nc.vector.tensor_add(
                            kT_sb[:, 0, 0:P], pss[1][:, 0:P],
                            bqkv_sb[:, 1:2].broadcast_to([P, P]),
                        )
                        nc.vector.tensor_add(
                            kT_sb[:, 0, P:qw], pss[1][:, P:qw],
                            bqkv_sb[:, 1:2].broadcast_to([P, qw - P]),
                        )

            with (
                tc.tile_pool(name="opool", bufs=4) as opool,
                tc.tile_pool(name="otpool", bufs=4) as otpool,
                tc.tile_pool(name="outpool", bufs=3) as outpool,
                tc.tile_pool(name="zrpool", bufs=10) as zrpool,
            ):
                TRUNC = int(os.environ.get("ATTN_TRUNC", "0"))
                exp_count = [0]

                def scores_exp(b, row0, nbb, mc):
                    """Scores + exp for one m-chunk -> es_arr[b%ES][:, mc]."""
                    if TRUNC and exp_count[0] >= TRUNC:
                        return
                    exp_count[0] += 1
                    nsl = slice(row0, row0 + nbb)
                    s_ps = sps.tile([P, 2 * nbb], f32, tag="s",
                                    name=f"s_ps_{b}_{mc}")
                    for h in range(HPC):
                        hsl = slice(h * DH, (h + 1) * DH)
                        if SCORES_FP8:
                            nc.tensor.matmul(
                                s_ps[:, h * nbb:(h + 1) * nbb],
                                kT_sb[hsl, :, mc * P:(mc + 1) * P],
                                qT_sb[hsl, :, nsl],
                                perf_mode=DR,
                            )
                        else:
                            nc.tensor.matmul(
                                s_ps[:, h * nbb:(h + 1) * nbb],
                                kT_sb[hsl, 0, mc * P:(mc + 1) * P],
                                qT_sb[hsl, 0, nsl],
                            )
                    nc.scalar.activation(es_arr[b % ES_BUFS][:, mc, :], s_ps[:],
                                         AF.Exp, scale=SCALE)

                # PV PSUM: one tile holds both heads of one q-chunk pair;
                # pre-memset + start=False so streams never clobber (PSUM
                # "start" zeroes whole banks on this target) and no
                # stream-to-stream WAR chaining through bank reuse.
                pv_tiles = {}

                def pv_qcpair_psum(b, qp):
                    pv = pvps.tile([P, 2 * HPC, DH + 1], f32, tag="pv",
                                   name=f"pvp_{b}_{qp}")
                    nc.vector.memset(pv[:], 0.0)
                    return pv

                def pv_stream(b, nbb, qc, h):
                    """One PV accumulation stream into its qc-pair tile."""
                    es = es_arr[b % ES_BUFS]
                    key = (b, qc // 2)
                    if key not in pv_tiles:
                        pv_tiles[key] = pv_qcpair_psum(b, qc // 2)
                    pv = pv_tiles[key][:, (qc % 2) * HPC + h, :]
                    for mc in range(m_chunks):
                        nc.tensor.matmul(
                            pv,
                            es[:, mc, h * nbb + qc * P:h * nbb + (qc + 1) * P],
                            v_sb[:, mc, h * (DH + 1):(h + 1) * (DH + 1)],
                            start=False,
                            stop=(mc == m_chunks - 1),
                            skip_group_check=True,
                        )
                    return pv

                def division(b, qc, h, pv, ov, on_act=False):
                    """O[:, h] = O'/rowsum (q-major). The reciprocal is DVE;
                    the mul can run on ACT (scaled copy, per-partition zr)
                    when DVE is the tail bottleneck."""
                    zr = zrpool.tile([P, 1], f32, tag="zr",
                                     name=f"zr_{b}_{qc}_{h}")
                    with nc.allow_low_precision(reason="softmax recip"):
                        nc.vector.reciprocal(zr[:], pv[:, DH:DH + 1])
                    if on_act:
                        nc.scalar.mul(ov[:, h, :], pv[:, 0:DH], zr[:])
                    else:
                        nc.vector.tensor_mul(
                            ov[:, h, :], pv[:, 0:DH],
                            zr[:].broadcast_to([P, DH]),
                        )

                def pv_div_qc(b, nbb, qc):
                    """Both heads' PV streams + divisions for one q-chunk."""
                    ov = opool.tile([P, HPC, DH], bf16, tag="o",
                                    name=f"o_{b}_{qc}")
                    for h in range(HPC):
                        pv = pv_stream(b, nbb, qc, h)
                        division(b, qc, h, pv, ov)
                    return ov

                def pv_mc_last(b, nbb, qp, mcs):
                    """mc-major PV for the last block: accumulate the given
                    m-chunks for all 4 streams of one qc-pair."""
                    es = es_arr[b % ES_BUFS]
                    key = (b, qp)
                    if key not in pv_tiles:
                        pv_tiles[key] = pv_qcpair_psum(b, qp)
                    pv = pv_tiles[key]
                    for mc in mcs:
                        for qi in range(2):
                            qc = qp * 2 + qi
                            for h in range(HPC):
                                nc.tensor.matmul(
                                    pv[:, qi * HPC + h, :],
                                    es[:, mc,
                                       h * nbb + qc * P:h * nbb + (qc + 1) * P],
                                    v_sb[:, mc,
                                         h * (DH + 1):(h + 1) * (DH + 1)],
                                    start=False,
                                    stop=(mc == m_chunks - 1),
                                    skip_group_check=True,
                                )

                def div_proj_last(b, row0, nbb):
                    """Tail: divisions + projections for the last block.

                    Emitted breadth-first (all recips, all muls, ...) with
                    qc-chains alternating DVE/ACT, so each engine streams
                    same-type items back-to-back and the four chains
                    pipeline instead of serializing."""
                    nqc = nbb // P
                    ovs, oTps, oTs = {}, {}, {}
                    # one reciprocal + one broadcast-mul per qc-PAIR tile
                    # (strided over the 4 stream rowsums) halves the number
                    # of cross-engine hops in the tail
                    for qp in range(nqc // 2):
                        pv = pv_tiles[(b, qp)]
                        zr = zrpool.tile([P, 2 * HPC, 1], f32, tag="zr4",
                                         name=f"zr_{b}_{qp}")
                        with nc.allow_low_precision(reason="softmax recip"):
                            nc.vector.reciprocal(zr[:], pv[:, :, DH:DH + 1])
                        ov = opool.tile([P, 2 * HPC, DH], bf16, tag="o",
                                        name=f"o_{b}_{qp}")
                        nc.vector.tensor_mul(
                            ov[:], pv[:, :, 0:DH],
                            zr[:].broadcast_to([P, 2 * HPC, DH]),
                        )
                        ovs[qp] = ov
                    # leftover queued projection rides here: after the tail
                    # divisions grabbed the DVE, before the PE transposes
                    while projq:
                        pq = projq.popleft()
                        project_qc(*pq[:3], pq[3])
                    for qc in range(nqc):
                        oTps[qc] = accp.tile([P, P], bf16, tag="acc",
                                             name=f"oT_ps_{b}_{qc}")
                        nc.tensor.transpose(
                            oTps[qc][:],
                            ovs[qc // 2][:, (qc % 2) * HPC:(qc % 2 + 1) * HPC, :],
                            ident[:],
                        )
                        oTs[qc] = otpool.tile([P, P], bf16, tag="oT",
                                              name=f"oT_{b}_{qc}")
                        cp = nc.scalar.copy if qc % 2 else nc.vector.tensor_copy
                        cp(oTs[qc][:], oTps[qc][:])
                    for qc in range(nqc):
                        on_act = qc % 2 == 1
                        cp = nc.scalar.copy if on_act else nc.vector.tensor_copy
                        out_sb = outpool.tile([P, D], f16, tag="out",
                                              name=f"out_{b}_{qc}")
                        row = row0 + qc * P
                        for half in range(2):
                            pp = accp.tile([P, 512], f32, tag="acc",
                                           name=f"pp_{b}_{qc}_{half}")
                            nc.tensor.matmul(
                                pp[:], oTs[qc][:],
                                wp_sb[:, half * 512:(half + 1) * 512],
                            )
                            dslc = out_sb[:, half * 512:(half + 1) * 512]
                            cp(dslc, pp[:])
                            nc.sync.dma_start(
                                out_d.ap()[row:row + P,
                                           half * 512:(half + 1) * 512],
                                dslc,
                            )

                def project_qc(b, row0, qc, ov, on_act=False):
                    """PE transpose + projection + drains + out DMA."""
                    cp = nc.scalar.copy if on_act else nc.vector.tensor_copy
                    oT_ps = accp.tile([P, P], bf16, tag="acc",
                                      name=f"oT_ps_{b}_{qc}")
                    nc.tensor.transpose(oT_ps[:], ov[:], ident[:])
                    oT = otpool.tile([P, P], bf16, tag="oT",
                                     name=f"oT_{b}_{qc}")
                    cp(oT[:], oT_ps[:])
                    out_sb = outpool.tile([P, D], f16, tag="out",
                                          name=f"out_{b}_{qc}")
                    row = row0 + qc * P
                    for half in range(2):
                        pp = accp.tile([P, 512], f32, tag="acc",
                                       name=f"pp_{b}_{qc}_{half}")
                        nc.tensor.matmul(
                            pp[:], oT[:],
                            wp_sb[:, half * 512:(half + 1) * 512],
                        )
                        dslc = out_sb[:, half * 512:(half + 1) * 512]
                        cp(dslc, pp[:])
                        nc.sync.dma_start(
                            out_d.ap()[row:row + P, half * 512:(half + 1) * 512],
                            dslc,
                        )

                # ================= main schedule =================
                assert nb == qw, "block size must match j-slice width"
                blocks = [nb] * n_blocks
                mcs_per_j = qw // P

                # PV/div/proj work queue: entries (b, row0, qc), popped into
                # later blocks' PE slack once all of v_sb has been emitted.
                from collections import deque
                pvq = deque()
                # pops per g for blocks 1..: v(j1..3) occupy b1's early slack
                pb = os.environ.get("ATTN_POPS", "0002,1222,2101")
                rows = [[int(c) for c in r] for r in pb.split(",")]
                pop_budget = {i + 1: rows[i] for i in range(len(rows))}

                PROBE = os.environ.get("ATTN_PROBE", "0") == "1"
                projq = deque()   # (b, row0, qc, ov) divided, awaiting proj

                def pop_pv(k, last=False):
                    """Pop k PV+division units, then one deferred projection.

                    Projections lag divisions by one pop so the DVE never
                    has a projection drain queued ahead of the division the
                    next PV stream's PSUM-slot reuse is waiting on."""
                    if PROBE:
                        pvq.clear()
                        return
                    for _ in range(k):
                        if not pvq:
                            break
                        pb, prow0, qc = pvq.popleft()
                        ov = pv_div_qc(pb, nb, qc)
                        projq.append((pb, prow0, qc, ov))
                    keep = 0 if last else 1
                    while len(projq) > keep:
                        pb, prow0, qc, ov = projq.popleft()
                        project_qc(pb, prow0, qc, ov)

                row0 = 0
                for b, nbb in enumerate(blocks):
                    if b == 0:
                        # interleave with QKV j-sweeps: k(j) gates scores of
                        # its m-chunks and is always the next x-slice off the
                        # wire - nothing else rides ahead of it on the PE;
                        # v(j0)/q(j1) fill the PE after the last k lands.
                        qkv_j0_qk()
                        for j in range(n_j):
                            if j > 0:
                                qkv_j(j, parts=(1,))
                            for mc in range(j * mcs_per_j, (j + 1) * mcs_per_j):
                                scores_exp(b, row0, nbb, mc)
                        qkv_j(0, parts=(2,))
                        qkv_j(1, parts=(0,))
                    elif b < n_blocks - 1:
                        # block b's scores/exp feed ACT; queued PV/div/proj
                        # (and leftover QKV v-parts) ride the PE slack
                        for g in range(4):
                            for mc in range(g * 4, g * 4 + 4):
                                scores_exp(b, row0, nbb, mc)
                            if b == 1 and g < 3:
                                qkv_j(g + 1, parts=(2,))
                            if g == 0 and b + 1 < n_blocks:
                                qkv_j(b + 1, parts=(0,))
                            pop_pv(pop_budget[b][g])
                    else:
                        # last block: mc-major PV rides along with the exp
                        # stream; each qc-pair starts only after the pops
                        # that free its PSUM slot (avoids a DVE-order
                        # deadlock on the memset's WAR)
                        for g in range(4):
                            for mc in range(g * 4, g * 4 + 4):
                                scores_exp(b, row0, nbb, mc)
                            pop_pv(pop_budget[b][g])
                            if g == 0:
                                pv_mc_last(b, nbb, 0, range(0, 4))
                            elif g == 1:
                                pv_mc_last(b, nbb, 0, range(4, 8))
                                pv_mc_last(b, nbb, 1, range(0, 8))
                            else:
                                pv_mc_last(b, nbb, 0, range(g * 4, g * 4 + 4))
                                pv_mc_last(b, nbb, 1, range(g * 4, g * 4 + 4))

                    if b < n_blocks - 1:
                        for qc in range(QC):
                            pvq.append((b, row0, qc))
                    row0 += nbb
                # flush any queue leftovers, then the last block's tail
                pop_pv(len(pvq), last=False)
                while projq:
                    pb, prow0, qc, ov = projq.popleft()
                    project_qc(pb, prow0, qc, ov)
                div_proj_last(n_blocks - 1, (n_blocks - 1) * nb, nb)

            pvps.release()
            sps.release()
            accp.release()

    nc.compile()
    return nc


def _host_prep(x, w_qkv, b_qkv, w_proj, n=N):
    """Per-core input maps (dtypes match the DRAM tensor declarations)."""
    import ml_dtypes

    bf = ml_dtypes.bfloat16
    xT = np.ascontiguousarray(x.T.astype(bf))
    in_maps = []
    for c in range(NCORES):
        wq = w_qkv[0 * D + c * P:0 * D + (c + 1) * P, :]
        wk = w_qkv[1 * D + c * P:1 * D + (c + 1) * P, :]
        wv = w_qkv[2 * D + c * P:2 * D + (c + 1) * P, :]
        # part-major [p, part, o, c]: contiguous per-part weight DMAs
        wqkvT = np.ascontiguousarray(
            np.stack(
                [a.T.reshape(D_CHUNKS, P, P).transpose(1, 0, 2) for a in (wq, wk, wv)],
                axis=1,
            ).astype(bf)
        )
        wpT = np.ascontiguousarray(w_proj[:, c * P:(c + 1) * P].T.astype(bf))
        bq = b_qkv[0 * D + c * P:0 * D + (c + 1) * P]
        bk = b_qkv[1 * D + c * P:1 * D + (c + 1) * P]
        bv = b_qkv[2 * D + c * P:2 * D + (c + 1) * P]
        bqkv = np.ascontiguousarray(
            np.stack([bq, bk, bv], axis=1).astype(np.float32)
        )
        bvrow = np.ascontiguousarray(bv.reshape(1, P).astype(bf))
        in_maps.append({"xT": xT, "wqkvT": wqkvT, "wpT": wpT, "bqkv": bqkv,
                        "bvrow": bvrow})
    return in_maps


_NC_CACHE = {}


def run(x, w_qkv, b_qkv, w_proj, b_proj, trace=False, n=N, nb=None, **spmd_kwargs):
    from concourse.bass_utils import run_bass_kernel_spmd

    if nb is None:
        nb = NB
    key = (n, nb, SCORES_FP8, WARMUP, ES_BUFS,
           os.environ.get("ATTN_POPS", ""), os.environ.get("ATTN_SPLIT_X0", ""))
    if key not in _NC_CACHE:
        _NC_CACHE[key] = _build_nc(n=n, nb=nb)
    nc = _NC_CACHE[key]

    in_maps = _host_prep(
        np.asarray(x), np.asarray(w_qkv), np.asarray(b_qkv), np.asarray(w_proj), n=n
    )
    results = run_bass_kernel_spmd(
        nc, in_maps, core_ids=list(range(NCORES)), trace=trace, **spmd_kwargs
    )
    acc = np.zeros((n, D), dtype=np.float64)
    for c in range(NCORES):
        acc += results.results[c]["out_part"].astype(np.float64)
    acc += np.asarray(b_proj).astype(np.float64)
    return acc.astype(np.float32), results


def kernel(x, w_qkv, b_qkv, w_proj, b_proj):
    out, _ = run(x, w_qkv, b_qkv, w_proj, b_proj, trace=False)
    return out


# revision 82
# speedup vs baseline: 1.3017x; 1.0055x over previous
"""Multi-head attention (N=2048, D=1024, H=16) on 8 TRN2 NeuronCores.

Sharding: tensor-parallel over heads (2 heads / core). x is replicated
(pre-transposed + pre-cast on host), each core computes QKV / scores /
softmax / PV / out-proj for its 2 heads, producing a partial (N, D)
projection output in fp16. The all-reduce over cores is the host-side
f64 sum of the 8 partials (+ b_proj), cast back to f32.

Device inputs (per core):
  xT      (D, N)    bf16 : x transposed (host prep)
  wqkvT   (128,3,8,128) bf16 : [Wq.T | Wk.T | Wv.T] slices, part-major
  wpT     (128, D)  bf16 : w_proj[:, core_cols].T
  bqkv    (128, 3)  f32  : [bq | bk | bv] slices
  out_part(N, D)    f16  : partial projection output

Per-core pipeline (ACT-bound design; exp on the scalar engine is the
64 x 1024-elem roofline, every other engine hides under it):
  QKV   bf16 matmuls j-sliced; x streams per j-slice (j0 split in two
        o-halves riding a reordered DMA queue), K(j) is always the next
        x-slice off the wire so block-0 scores are never queued behind
        lower-priority PE work. V is computed seq-major directly
        (lhsT = x chunk, rhs = wv chunk) into [V_h0|1|V_h1|1]; its bias
        rides a K=1 ones-row matmul (exact through the softmax mean).
  scores lhsT = K.T slice, rhs = Q.T (bf16; fp8e4-DoubleRow knob exists
        but costs ~2.4e-2 rel err - over the gate).
  exp   one ACT pass per m-chunk into per-block es arrays (bufs=4).
  PV    deferred one-or-more blocks behind exp via a work queue popped
        into later blocks' PE slack: per (qc, h) stream, lhsT = es chunk
        (128 keys x 128 q), rhs = [V|1] (128 keys x 65, ap_size 65).
        PSUM accumulators are pre-memset and all matmuls use
        start=False (PSUM "start" zeroes whole banks on this target),
        so streams share banks without clobbering and without WAR
        chains. The last block accumulates mc-major, riding along its
        own exp stream so only division remains after the last exp.
  div   rowsum is column 64 (q-major): DVE reciprocal + per-partition
        broadcast mul -> O bf16. PE transpose -> O.T.
  proj  O.T^T @ wpT in bf16 -> fp16 partial, one DMA per 128x512 chunk.
        The last block's division/projection chains alternate DVE/ACT
        and are emitted breadth-first to pipeline the tail.
"""

import os
import sys

import numpy as np

for _p in ("/opt/trn_rl_repo",):
    if os.path.isdir(_p) and _p not in sys.path:
        sys.path.insert(0, _p)

N, D, H = 2048, 1024, 16
DH = D // H                 # 64
NCORES = 8
HPC = H // NCORES           # 2 heads per core
P = 128
SCALE = 1.0 / DH ** 0.5

D_CHUNKS = D // P           # 8

# config knobs (overridable before run() for experiments)
SCORES_FP8 = os.environ.get("ATTN_SCORES_FP8", "0") == "1"
NB = int(os.environ.get("ATTN_NB", "512"))                 # query-block size
WARMUP = int(os.environ.get("ATTN_WARMUP", "24"))          # PE warmup transposes
ES_BUFS = int(os.environ.get("ATTN_ES_BUFS", "4"))         # es block arrays


def _build_nc(n=N, nb=NB):
    """Build the per-core Bass module (SPMD: identical program, per-core data)."""
    import concourse.bass as bass  # noqa: F401
    import concourse.mybir as mybir
    import concourse.tile as tile
    from concourse import bacc
    from concourse.masks import make_identity

    f32 = mybir.dt.float32
    bf16 = mybir.dt.bfloat16
    f16 = mybir.dt.float16
    fp8 = mybir.dt.float8e4
    AF = mybir.ActivationFunctionType
    DR = mybir.MatmulPerfMode.DoubleRow

    m_chunks = n // P           # 16
    n_blocks = n // nb          # 4
    QC = nb // P                # 4 query chunks per block

    nc = bacc.Bacc(
        "TRN2",
        target_bir_lowering=False,
        debug=False,
        enable_asserts=True,
        num_devices=NCORES,
    )

    xT_d = nc.dram_tensor("xT", (D, n), bf16, kind="ExternalInput")
    wqkvT_d = nc.dram_tensor("wqkvT", (P, 3, D_CHUNKS, P), bf16, kind="ExternalInput")
    wpT_d = nc.dram_tensor("wpT", (P, D), bf16, kind="ExternalInput")
    bqkv_d = nc.dram_tensor("bqkv", (P, 3), f32, kind="ExternalInput")
    bvrow_d = nc.dram_tensor("bvrow", (1, P), bf16, kind="ExternalInput")
    out_d = nc.dram_tensor("out_part", (n, D), f16, kind="ExternalOutput")

    with tile.TileContext(nc) as tc:
        with (
            tc.tile_pool(name="consts", bufs=1) as consts,
            tc.tile_pool(name="xpool", bufs=1) as xpool,
            tc.tile_pool(name="qkpool", bufs=1) as qkpool,
        ):
            # ---- inputs ----
            wqkv_sb = consts.tile([P, 3, D_CHUNKS, P], bf16)
            wp_sb = consts.tile([P, D], bf16)
            bqkv_sb = consts.tile([P, 3], f32)
            xT_sb = xpool.tile([P, D_CHUNKS, n], bf16)

            qw = min(512, n)
            n_j = n // qw
            # one DMA per j-slice of x, except j0 which is split in two
            # o-halves so q/k start accumulating while half 2 streams in;
            # order: x-h1, wq, wk, x-h2 puts the startup chain on the
            # critical path exactly once
            xT_re = xT_d.ap().rearrange("(o p) x -> p o x", p=P)

            def x_slice(j, o0=0, o1=D_CHUNKS):
                nc.sync.dma_start(
                    xT_sb[:, o0:o1, j * qw:(j + 1) * qw],
                    xT_re[:, o0:o1, j * qw:(j + 1) * qw],
                )

            SPLIT_X0 = os.environ.get("ATTN_SPLIT_X0", "1") == "1"
            if SPLIT_X0:
                # startup chain: q's o0:4 matmuls only need x-h1 + wq-h1,
                # so the weight DMAs are split per o-half and interleaved
                x_slice(0, 0, 4)
                nc.sync.dma_start(wqkv_sb[:, 0, 0:4], wqkvT_d.ap()[:, 0, 0:4])
                nc.sync.dma_start(wqkv_sb[:, 0, 4:8], wqkvT_d.ap()[:, 0, 4:8])
                x_slice(0, 4, 8)
                nc.sync.dma_start(wqkv_sb[:, 1, 0:4], wqkvT_d.ap()[:, 1, 0:4])
                nc.sync.dma_start(wqkv_sb[:, 1, 4:8], wqkvT_d.ap()[:, 1, 4:8])
            else:
                nc.sync.dma_start(wqkv_sb[:, 0], wqkvT_d.ap()[:, 0])
                nc.sync.dma_start(wqkv_sb[:, 1], wqkvT_d.ap()[:, 1])
                x_slice(0)
            nc.sync.dma_start(bqkv_sb[:], bqkv_d.ap())
            nc.sync.dma_start(wqkv_sb[:, 2], wqkvT_d.ap()[:, 2])
            for j in range(1, n_j):
                x_slice(j, 0, 4)
                x_slice(j, 4, 8)
            nc.sync.dma_start(wp_sb[:], wpT_d.ap())

            ident = consts.tile([P, P], bf16)
            make_identity(nc, ident[:])

            # ---- persistent activations ----
            if SCORES_FP8:
                # [p, member, col]: member 1 is an all-zero DoubleRow partner
                qT_sb = qkpool.tile([P, 2, n], fp8)
                kT_sb = qkpool.tile([P, 2, n], fp8)
                nc.gpsimd.memset(qT_sb[:, 1, 0:qw], 0.0)
                nc.gpsimd.memset(kT_sb[:, 1, 0:qw], 0.0)
                nc.gpsimd.memset(qT_sb[:, 1, qw:n], 0.0)
                nc.gpsimd.memset(kT_sb[:, 1, qw:n], 0.0)
            else:
                qT_sb = qkpool.tile([P, 1, n], bf16)
                kT_sb = qkpool.tile([P, 1, n], bf16)
            # [V_h0|1|V_h1|1] seq-major, per m-chunk
            v_sb = qkpool.tile([P, m_chunks, 2 * (DH + 1)], bf16)
            nc.gpsimd.memset(v_sb[:, :, DH:DH + 1], 1.0)
            nc.gpsimd.memset(v_sb[:, :, 2 * DH + 1:2 * DH + 2], 1.0)
            # K=1 ones row + bv row: folds the V bias into the PV average
            # (sum_k p_k (v+bv) = O + bv exactly, rowsum column unaffected)
            ones_row = consts.tile([P, P], bf16)
            nc.gpsimd.memset(ones_row[0:1, :], 1.0)
            bvrow_sb = consts.tile([P, P], bf16)
            nc.sync.dma_start(bvrow_sb[0:1, :], bvrow_d.ap())
            # es arrays: one full block of exp(S) per buffer; 3 bufs so the
            # exp stream of block b never waits on PV of block b-ES_BUFS+1
            es_arr = [
                qkpool.tile([P, m_chunks, 2 * nb], bf16, name=f"es_arr{i}")
                for i in range(ES_BUFS)
            ]

            # ===== PSUM pools (8 banks): sps 2x2 + pvps 2x1 + accp 2x1 =====
            accp = tc.alloc_tile_pool(name="accp", bufs=2, space="PSUM")
            sps = tc.alloc_tile_pool(name="sps", bufs=2, space="PSUM")
            pvps = tc.alloc_tile_pool(name="pvps", bufs=2, space="PSUM")

            for wi in range(WARMUP):
                wt = accp.tile([P, P], bf16, tag="acc", name=f"warm_{wi}")
                nc.tensor.transpose(wt[:], ident[:], ident[:])

            dst = {0: qT_sb, 1: kT_sb}

            SKIP_PARTS = set(
                int(c) for c in os.environ.get("ATTN_SKIP_PARTS", "")
                if c.isdigit()
            )

            def qkv_j(j, parts):
                for part in parts:
                    if part in SKIP_PARTS:
                        continue
                    if part == 2:
                        # V computed seq-major directly: lhsT = x chunk
                        # (stationary), rhs = wv chunk -> out [128 seq, 128
                        # dh]; the K=1 ones-row matmul folds in the V bias.
                        # One PSUM bank holds all 4 m-chunks of the j-slice:
                        # pre-memset + start=False accumulation everywhere
                        # (PSUM "start" zeroes whole banks on this target).
                        nmc = qw // P
                        mc0 = j * qw // P
                        vp = accp.tile([P, nmc, P], f32, tag="acc",
                                       name=f"v_ps_{j}")
                        nc.vector.memset(vp[:], 0.0)
                        for i in range(nmc):
                            msl = slice((mc0 + i) * P, (mc0 + i + 1) * P)
                            for o in range(D_CHUNKS):
                                nc.tensor.matmul(
                                    vp[:, i, :],
                                    xT_sb[:, o, msl],
                                    wqkv_sb[:, 2, o, :],
                                    start=False, stop=False,
                                    skip_group_check=True,
                                )
                            nc.tensor.matmul(
                                vp[:, i, :], ones_row[0:1, :], bvrow_sb[0:1, :],
                                start=False, stop=(i == nmc - 1),
                                skip_group_check=True,
                            )
                        nc.vector.tensor_copy(
                            v_sb[:, mc0:mc0 + nmc, 0:DH], vp[:, :, 0:DH]
                        )
                        nc.vector.tensor_copy(
                            v_sb[:, mc0:mc0 + nmc, DH + 1:2 * DH + 1],
                            vp[:, :, DH:2 * DH],
                        )
                        continue
                    ps = accp.tile([P, qw], f32, tag="acc",
                                   name=f"qkv_ps_{part}_{j}")
                    for o in range(D_CHUNKS):
                        nc.tensor.matmul(
                            ps[:], wqkv_sb[:, part, o, :],
                            xT_sb[:, o, j * qw:(j + 1) * qw],
                            start=(o == 0), stop=(o == D_CHUNKS - 1),
                        )
                    with nc.allow_low_precision(reason="qk fp8 drain"):
                        if part == 1:
                            # split the K drain so the slice's first m-chunks
                            # unblock their scores before the full drain
                            for c0, c1 in ((0, P), (P, qw)):
                                nc.vector.tensor_add(
                                    dst[part][:, 0, j * qw + c0:j * qw + c1],
                                    ps[:, c0:c1],
                                    bqkv_sb[:, part:part + 1]
                                    .broadcast_to([P, c1 - c0]),
                                )
                        else:
                            jsl = slice(j * qw, (j + 1) * qw)
                            nc.vector.tensor_add(
                                dst[part][:, 0, jsl], ps[:],
                                bqkv_sb[:, part:part + 1].broadcast_to([P, qw]),
                            )

            def qkv_j0_qk():
                """q+k for j0, interleaved two-phase over the split x DMA;
                k's drain is split so scores(mc0) starts on the first piece."""
                pss = {}
                for part in (0, 1):
                    pss[part] = accp.tile([P, qw], f32, tag="acc",
                                          name=f"qkv_ps_{part}_0")
                for part in (0, 1):
                    if part in SKIP_PARTS:
                        continue
                    for o in range(D_CHUNKS):
                        nc.tensor.matmul(
                            pss[part][:], wqkv_sb[:, part, o, :],
                            xT_sb[:, o, 0:qw],
                            start=(o == 0), stop=(o == D_CHUNKS - 1),
                        )
                with nc.allow_low_precision(reason="qk fp8 drain"):
                    if 0 not in SKIP_PARTS:
                        nc.vector.tensor_add(
                            qT_sb[:, 0, 0:qw], pss[0][:],
                            bqkv_sb[:, 0:1].broadcast_to([P, qw]),
                        )
                    if 1 not in SKIP_PARTS:
                        nc.vector.tensor_add(
                            kT_sb[:, 0, 0:P], pss[1][:, 0:P],
                            bqkv_sb[:, 1:2].broadcast_to([P, P]),
                        )
                        nc.vector.tensor_add(
                            kT_sb[:, 0, P:qw], pss[1][:, P:qw],
                            bqkv_sb[:, 1:2].broadcast_to([P, qw - P]),
                        )

            with (
                tc.tile_pool(name="opool", bufs=4) as opool,
                tc.tile_pool(name="otpool", bufs=4) as otpool,
                tc.tile_pool(name="outpool", bufs=5) as outpool,
                tc.tile_pool(name="zrpool", bufs=10) as zrpool,
            ):
                TRUNC = int(os.environ.get("ATTN_TRUNC", "0"))
                exp_count = [0]

                def scores_exp(b, row0, nbb, mc):
                    """Scores + exp for one m-chunk -> es_arr[b%ES][:, mc]."""
                    if TRUNC and exp_count[0] >= TRUNC:
                        return
                    exp_count[0] += 1
                    nsl = slice(row0, row0 + nbb)
                    s_ps = sps.tile([P, 2 * nbb], f32, tag="s",
                                    name=f"s_ps_{b}_{mc}")
                    for h in range(HPC):
                        hsl = slice(h * DH, (h + 1) * DH)
                        if SCORES_FP8:
                            nc.tensor.matmul(
                                s_ps[:, h * nbb:(h + 1) * nbb],
                                kT_sb[hsl, :, mc * P:(mc + 1) * P],
                                qT_sb[hsl, :, nsl],
                                perf_mode=DR,
                            )
                        else:
                            nc.tensor.matmul(
                                s_ps[:, h * nbb:(h + 1) * nbb],
                                kT_sb[hsl, 0, mc * P:(mc + 1) * P],
                                qT_sb[hsl, 0, nsl],
                            )
                    nc.scalar.activation(es_arr[b % ES_BUFS][:, mc, :], s_ps[:],
                                         AF.Exp, scale=SCALE)

                # PV PSUM: one tile holds both heads of one q-chunk pair;
                # pre-memset + start=False so streams never clobber (PSUM
                # "start" zeroes whole banks on this target) and no
                # stream-to-stream WAR chaining through bank reuse.
                pv_tiles = {}

                def pv_qcpair_psum(b, qp):
                    pv = pvps.tile([P, 2 * HPC, DH + 1], f32, tag="pv",
                                   name=f"pvp_{b}_{qp}")
                    nc.vector.memset(pv[:], 0.0)
                    return pv

                def pv_stream(b, nbb, qc, h):
                    """One PV accumulation stream into its qc-pair tile."""
                    es = es_arr[b % ES_BUFS]
                    key = (b, qc // 2)
                    if key not in pv_tiles:
                        pv_tiles[key] = pv_qcpair_psum(b, qc // 2)
                    pv = pv_tiles[key][:, (qc % 2) * HPC + h, :]
                    for mc in range(m_chunks):
                        nc.tensor.matmul(
                            pv,
                            es[:, mc, h * nbb + qc * P:h * nbb + (qc + 1) * P],
                            v_sb[:, mc, h * (DH + 1):(h + 1) * (DH + 1)],
                            start=False,
                            stop=(mc == m_chunks - 1),
                            skip_group_check=True,
                        )
                    return pv

                def division(b, qc, h, pv, ov, on_act=False):
                    """O[:, h] = O'/rowsum (q-major). The reciprocal is DVE;
                    the mul can run on ACT (scaled copy, per-partition zr)
                    when DVE is the tail bottleneck."""
                    zr = zrpool.tile([P, 1], f32, tag="zr",
                                     name=f"zr_{b}_{qc}_{h}")
                    with nc.allow_low_precision(reason="softmax recip"):
                        nc.vector.reciprocal(zr[:], pv[:, DH:DH + 1])
                    if on_act:
                        nc.scalar.mul(ov[:, h, :], pv[:, 0:DH], zr[:])
                    else:
                        nc.vector.tensor_mul(
                            ov[:, h, :], pv[:, 0:DH],
                            zr[:].broadcast_to([P, DH]),
                        )

                def pv_div_qc(b, nbb, qc):
                    """Both heads' PV streams + divisions for one q-chunk."""
                    ov = opool.tile([P, HPC, DH], bf16, tag="o",
                                    name=f"o_{b}_{qc}")
                    for h in range(HPC):
                        pv = pv_stream(b, nbb, qc, h)
                        division(b, qc, h, pv, ov)
                    return ov

                def pv_mc_last(b, nbb, qp, mcs):
                    """mc-major PV for the last block: accumulate the given
                    m-chunks for all 4 streams of one qc-pair."""
                    es = es_arr[b % ES_BUFS]
                    key = (b, qp)
                    if key not in pv_tiles:
                        pv_tiles[key] = pv_qcpair_psum(b, qp)
                    pv = pv_tiles[key]
                    for mc in mcs:
                        for qi in range(2):
                            qc = qp * 2 + qi
                            for h in range(HPC):
                                nc.tensor.matmul(
                                    pv[:, qi * HPC + h, :],
                                    es[:, mc,
                                       h * nbb + qc * P:h * nbb + (qc + 1) * P],
                                    v_sb[:, mc,
                                         h * (DH + 1):(h + 1) * (DH + 1)],
                                    start=False,
                                    stop=(mc == m_chunks - 1),
                                    skip_group_check=True,
                                )

                def div_proj_last(b, row0, nbb):
                    """Tail: divisions + projections for the last block.

                    Emitted breadth-first (all recips, all muls, ...) with
                    qc-chains alternating DVE/ACT, so each engine streams
                    same-type items back-to-back and the four chains
                    pipeline instead of serializing."""
                    nqc = nbb // P
                    ovs, oTps, oTs = {}, {}, {}
                    # one reciprocal + one broadcast-mul per qc-PAIR tile
                    # (strided over the 4 stream rowsums) halves the number
                    # of cross-engine hops in the tail
                    for qp in range(nqc // 2):
                        pv = pv_tiles[(b, qp)]
                        zr = zrpool.tile([P, 2 * HPC, 1], f32, tag="zr4",
                                         name=f"zr_{b}_{qp}")
                        with nc.allow_low_precision(reason="softmax recip"):
                            nc.vector.reciprocal(zr[:], pv[:, :, DH:DH + 1])
                        ov = opool.tile([P, 2 * HPC, DH], bf16, tag="o",
                                        name=f"o_{b}_{qp}")
                        nc.vector.tensor_mul(
                            ov[:], pv[:, :, 0:DH],
                            zr[:].broadcast_to([P, 2 * HPC, DH]),
                        )
                        ovs[qp] = ov
                    # leftover queued projection rides here: after the tail
                    # divisions grabbed the DVE, before the PE transposes
                    while projq:
                        pq = projq.popleft()
                        project_qc(*pq[:3], pq[3])
                    for qc in range(nqc):
                        oTps[qc] = accp.tile([P, P], bf16, tag="acc",
                                             name=f"oT_ps_{b}_{qc}")
                        nc.tensor.transpose(
                            oTps[qc][:],
                            ovs[qc // 2][:, (qc % 2) * HPC:(qc % 2 + 1) * HPC, :],
                            ident[:],
                        )
                        oTs[qc] = otpool.tile([P, P], bf16, tag="oT",
                                              name=f"oT_{b}_{qc}")
                        cp = nc.scalar.copy if qc % 2 else nc.vector.tensor_copy
                        cp(oTs[qc][:], oTps[qc][:])
                    for qc in range(nqc):
                        on_act = qc in (1, 2)
                        cp = nc.scalar.copy if on_act else nc.vector.tensor_copy
                        out_sb = outpool.tile([P, D], f16, tag="out",
                                              name=f"out_{b}_{qc}")
                        row = row0 + qc * P
                        for half in range(2):
                            # qc0/1's proj accumulators borrow the pvps
                            # banks (free after the divisions) so the tail
                            # projections pipeline over 4 PSUM slots
                            pool_ = pvps if qc < 2 else accp
                            pp = pool_.tile([P, 512], f32,
                                            tag="pv" if qc < 2 else "acc",
                                            name=f"pp_{b}_{qc}_{half}")
                            nc.tensor.matmul(
                                pp[:], oTs[qc][:],
                                wp_sb[:, half * 512:(half + 1) * 512],
                            )
                            dslc = out_sb[:, half * 512:(half + 1) * 512]
                            cp(dslc, pp[:])
                            nc.sync.dma_start(
                                out_d.ap()[row:row + P,
                                           half * 512:(half + 1) * 512],
                                dslc,
                            )

                def project_qc(b, row0, qc, ov, on_act=False):
                    """PE transpose + projection + drains + out DMA."""
                    cp = nc.scalar.copy if on_act else nc.vector.tensor_copy
                    oT_ps = accp.tile([P, P], bf16, tag="acc",
                                      name=f"oT_ps_{b}_{qc}")
                    nc.tensor.transpose(oT_ps[:], ov[:], ident[:])
                    oT = otpool.tile([P, P], bf16, tag="oT",
                                     name=f"oT_{b}_{qc}")
                    cp(oT[:], oT_ps[:])
                    out_sb = outpool.tile([P, D], f16, tag="out",
                                          name=f"out_{b}_{qc}")
                    row = row0 + qc * P
                    for half in range(2):
                        pp = accp.tile([P, 512], f32, tag="acc",
                                       name=f"pp_{b}_{qc}_{half}")
                        nc.tensor.matmul(
                            pp[:], oT[:],
                            wp_sb[:, half * 512:(half + 1) * 512],
                        )
                        dslc = out_sb[:, half * 512:(half + 1) * 512]
                        cp(dslc, pp[:])
                        nc.sync.dma_start(
                            out_d.ap()[row:row + P, half * 512:(half + 1) * 512],
                            dslc,
                        )

                # ================= main schedule =================
                assert nb == qw, "block size must match j-slice width"
                blocks = [nb] * n_blocks
                mcs_per_j = qw // P

                # PV/div/proj work queue: entries (b, row0, qc), popped into
                # later blocks' PE slack once all of v_sb has been emitted.
                from collections import deque
                pvq = deque()
                # pops per g for blocks 1..: v(j1..3) occupy b1's early slack
                pb = os.environ.get("ATTN_POPS", "0002,1222,2101")
                rows = [[int(c) for c in r] for r in pb.split(",")]
                pop_budget = {i + 1: rows[i] for i in range(len(rows))}

                PROBE = os.environ.get("ATTN_PROBE", "0") == "1"
                projq = deque()   # (b, row0, qc, ov) divided, awaiting proj

                def pop_pv(k, last=False):
                    """Pop k PV+division units, then one deferred projection.

                    Projections lag divisions by one pop so the DVE never
                    has a projection drain queued ahead of the division the
                    next PV stream's PSUM-slot reuse is waiting on."""
                    if PROBE:
                        pvq.clear()
                        return
                    for _ in range(k):
                        if not pvq:
                            break
                        pb, prow0, qc = pvq.popleft()
                        ov = pv_div_qc(pb, nb, qc)
                        projq.append((pb, prow0, qc, ov))
                    keep = 0 if last else 1
                    while len(projq) > keep:
                        pb, prow0, qc, ov = projq.popleft()
                        project_qc(pb, prow0, qc, ov)

                row0 = 0
                for b, nbb in enumerate(blocks):
                    if b == 0:
                        # interleave with QKV j-sweeps: k(j) gates scores of
                        # its m-chunks and is always the next x-slice off the
                        # wire - nothing else rides ahead of it on the PE;
                        # v(j0)/q(j1) fill the PE after the last k lands.
                        qkv_j0_qk()
                        for j in range(n_j):
                            if j > 0:
                                qkv_j(j, parts=(1,))
                            for mc in range(j * mcs_per_j, (j + 1) * mcs_per_j):
                                scores_exp(b, row0, nbb, mc)
                        qkv_j(0, parts=(2,))
                        qkv_j(1, parts=(0,))
                    elif b < n_blocks - 1:
                        # block b's scores/exp feed ACT; queued PV/div/proj
                        # (and leftover QKV v-parts) ride the PE slack
                        for g in range(4):
                            for mc in range(g * 4, g * 4 + 4):
                                scores_exp(b, row0, nbb, mc)
                            if b == 1 and g < 3:
                                qkv_j(g + 1, parts=(2,))
                            if g == 0 and b + 1 < n_blocks:
                                qkv_j(b + 1, parts=(0,))
                            pop_pv(pop_budget[b][g])
                    else:
                        # last block: mc-major PV rides along with the exp
                        # stream; each qc-pair starts only after the pops
                        # that free its PSUM slot (avoids a DVE-order
                        # deadlock on the memset's WAR)
                        for g in range(4):
                            for mc in range(g * 4, g * 4 + 4):
                                scores_exp(b, row0, nbb, mc)
                            pop_pv(pop_budget[b][g])
                            if g == 0:
                                pv_mc_last(b, nbb, 0, range(0, 4))
                            elif g == 1:
                                pv_mc_last(b, nbb, 0, range(4, 8))
                                pv_mc_last(b, nbb, 1, range(0, 8))
                            else:
                                pv_mc_last(b, nbb, 0, range(g * 4, g * 4 + 4))
                                pv_mc_last(b, nbb, 1, range(g * 4, g * 4 + 4))

                    if b < n_blocks - 1:
                        for qc in range(QC):
                            pvq.append((b, row0, qc))
                    row0 += nbb
                # flush any queue leftovers, then the last block's tail
                pop_pv(len(pvq), last=False)
                while projq:
                    pb, prow0, qc, ov = projq.popleft()
                    project_qc(pb, prow0, qc, ov)
                div_proj_last(n_blocks - 1, (n_blocks - 1) * nb, nb)

            pvps.release()
            sps.release()
            accp.release()

    nc.compile()
    return nc


def _host_prep(x, w_qkv, b_qkv, w_proj, n=N):
    """Per-core input maps (dtypes match the DRAM tensor declarations)."""
    import ml_dtypes

    bf = ml_dtypes.bfloat16
    xT = np.ascontiguousarray(x.T.astype(bf))
    in_maps = []
    for c in range(NCORES):
        wq = w_qkv[0 * D + c * P:0 * D + (c + 1) * P, :]
        wk = w_qkv[1 * D + c * P:1 * D + (c + 1) * P, :]
        wv = w_qkv[2 * D + c * P:2 * D + (c + 1) * P, :]
        # part-major [p, part, o, c]: contiguous per-part weight DMAs
        wqkvT = np.ascontiguousarray(
            np.stack(
                [a.T.reshape(D_CHUNKS, P, P).transpose(1, 0, 2) for a in (wq, wk, wv)],
                axis=1,
            ).astype(bf)
        )
        wpT = np.ascontiguousarray(w_proj[:, c * P:(c + 1) * P].T.astype(bf))
        bq = b_qkv[0 * D + c * P:0 * D + (c + 1) * P]
        bk = b_qkv[1 * D + c * P:1 * D + (c + 1) * P]
        bv = b_qkv[2 * D + c * P:2 * D + (c + 1) * P]
        bqkv = np.ascontiguousarray(
            np.stack([bq, bk, bv], axis=1).astype(np.float32)
        )
        bvrow = np.ascontiguousarray(bv.reshape(1, P).astype(bf))
        in_maps.append({"xT": xT, "wqkvT": wqkvT, "wpT": wpT, "bqkv": bqkv,
                        "bvrow": bvrow})
    return in_maps


_NC_CACHE = {}


def run(x, w_qkv, b_qkv, w_proj, b_proj, trace=False, n=N, nb=None, **spmd_kwargs):
    from concourse.bass_utils import run_bass_kernel_spmd

    if nb is None:
        nb = NB
    key = (n, nb, SCORES_FP8, WARMUP, ES_BUFS,
           os.environ.get("ATTN_POPS", ""), os.environ.get("ATTN_SPLIT_X0", ""))
    if key not in _NC_CACHE:
        _NC_CACHE[key] = _build_nc(n=n, nb=nb)
    nc = _NC_CACHE[key]

    in_maps = _host_prep(
        np.asarray(x), np.asarray(w_qkv), np.asarray(b_qkv), np.asarray(w_proj), n=n
    )
    results = run_bass_kernel_spmd(
        nc, in_maps, core_ids=list(range(NCORES)), trace=trace, **spmd_kwargs
    )
    acc = np.zeros((n, D), dtype=np.float64)
    for c in range(NCORES):
        acc += results.results[c]["out_part"].astype(np.float64)
    acc += np.asarray(b_proj).astype(np.float64)
    return acc.astype(np.float32), results


def kernel(x, w_qkv, b_qkv, w_proj, b_proj):
    out, _ = run(x, w_qkv, b_qkv, w_proj, b_proj, trace=False)
    return out
